# revision 1
# baseline (speedup 1.0000x reference)
"""Neighbor aggregation (gnn message passing) Bass kernel for Trainium2.

out[b, i] = sum_{e: src[e]==i} w[e] * H[b, dst[e]]   (per batch b)

8 NeuronCores: core = 2*b + s handles batch b, src-half s (output rows
[s*25000, (s+1)*25000)).  Edges are partitioned host-side by dst-half into two
phases so gather indices fit int16 after rebasing.  Per 1024-token chunk:
SWDGE dma_gather (HBM H rows -> SBUF token-major), DVE broadcast multiply by
w, SWDGE dma_scatter_add (CCE f32) into a parity-split SBUF accumulator.

Hardware constraints found by probing:
 - SWDGE gather/scatter calls are limited to 1024 tokens (64 descriptors per
   DMA engine per packet); larger calls crash the device.
 - dma_scatter_add loses read-modify-write updates when the same destination
   row appears twice in close proximity within one call, so the host packs
   tokens into chunks with UNIQUE src per chunk (round-aligned bins) and pads
   with a junk accumulator row (>= 25000) whose updates are discarded.
"""

import os
import sys

sys.path.insert(0, "/opt/trn_rl_repo")

import numpy as np

import concourse.bacc as bacc
import concourse.mybir as mybir
import concourse.tile as tile
from concourse.bass_utils import run_bass_kernel_spmd

B, N, E, HS = 4, 50000, 800000, 64
NHALF = N // 2                  # 25000
C = 1024                        # tokens per chunk (hard HW limit per SWDGE call)
NGRP = 98                       # parity groups: accumulator covers idx < 25088
PAD_ROW = 25080                 # junk accumulator row for padding tokens

LAST_RESULT = {}


def build(nc, ch_per_phase, n_nodes, nhalf, c, ngrp, hs):
    f32 = mybir.dt.float32
    i16 = mybir.dt.int16

    h_d = nc.dram_tensor("h", [n_nodes, hs], f32, kind="ExternalInput")
    gidx_d = nc.dram_tensor(
        "gidx", [2, ch_per_phase, 128, c // 16], i16, kind="ExternalInput"
    )
    sidx_d = nc.dram_tensor(
        "sidx", [2, ch_per_phase, 128, c // 16], i16, kind="ExternalInput"
    )
    wl_d = nc.dram_tensor(
        "wl", [2, ch_per_phase, 128, c // 128], f32, kind="ExternalInput"
    )
    acc_d = nc.dram_tensor("acc", [2, 2, 128, ngrp, hs], f32, kind="ExternalOutput")

    with tile.TileContext(nc) as tc:
        with tc.tile_pool(name="accp", bufs=1) as accp, \
             tc.tile_pool(name="work", bufs=4) as wp:
            accs = []
            for pr in range(2):
                a0 = accp.tile([128, ngrp, hs], f32, tag=f"acc{pr}0")
                a1 = accp.tile([128, ngrp, hs], f32, tag=f"acc{pr}1")
                nc.vector.memset(a0[:], 0.0)
                nc.vector.memset(a1[:], 0.0)
                accs.append((a0, a1))

            for phase in range(2):
                h_slice = h_d[:][phase * nhalf:(phase + 1) * nhalf, :]
                for k in range(ch_per_phase):
                    gi = wp.tile([128, c // 16], i16, tag="gi")
                    si = wp.tile([128, c // 16], i16, tag="si")
                    wt = wp.tile([128, c // 128], f32, tag="wt")
                    nc.sync.dma_start(gi[:], gidx_d[phase, k])
                    nc.sync.dma_start(si[:], sidx_d[phase, k])
                    nc.sync.dma_start(wt[:], wl_d[phase, k])

                    msgs = wp.tile([128, c // 128, hs], f32, tag="msgs")
                    nc.gpsimd.dma_gather(
                        out_ap=msgs[:],
                        in_ap=h_slice,
                        idxs_ap=gi[:],
                        num_idxs=c,
                        num_idxs_reg=c,
                        elem_size=hs,
                    )
                    nc.vector.tensor_tensor(
                        out=msgs[:],
                        in0=msgs[:],
                        in1=wt[:].unsqueeze(2).broadcast_to([128, c // 128, hs]),
                        op=mybir.AluOpType.mult,
                    )
                    a0, a1 = accs[k % 2]
                    nc.gpsimd.dma_scatter_add(
                        out_ap=a0[:],
                        in_ap=msgs[:],
                        idxs_ap=si[:],
                        num_idxs=c,
                        num_idxs_reg=c,
                        elem_size=hs,
                        sbuf_tokens_per_rank=128,
                        parity_reg=0,
                        out_ap_other=a1[:],
                    )

            for pr in range(2):
                nc.sync.dma_start(acc_d[pr, 0], accs[pr][0][:])
                nc.sync.dma_start(acc_d[pr, 1], accs[pr][1][:])
    return nc


_COMPILED = {}


def _get_compiled(ch_per_phase):
    if ch_per_phase not in _COMPILED:
        nc = bacc.Bacc("TRN2", target_bir_lowering=False, debug=False)
        build(nc, ch_per_phase, N, NHALF, C, NGRP, HS)
        nc.compile()
        _COMPILED[ch_per_phase] = nc
    return _COMPILED[ch_per_phase]


def _wrap16(idx, ch, c):
    a = idx.reshape(ch, c // 16, 16).transpose(0, 2, 1).astype(np.int16)
    return np.ascontiguousarray(np.tile(a, (1, 8, 1)))


def _round_pack(srcs, dsts, ws, cap):
    """Order tokens so equal src never share a 1024-token chunk: tokens get a
    within-src rank (round); each round starts at a fresh chunk boundary.
    Returns (g, s, w) arrays of length n_chunks*cap with pads."""
    order = np.argsort(srcs, kind="stable")
    ss = srcs[order]
    # within-group rank
    n = ss.shape[0]
    if n == 0:
        return (np.zeros(cap, np.int64), np.full(cap, PAD_ROW, np.int64),
                np.zeros(cap, np.float32), 1)
    first = np.r_[True, ss[1:] != ss[:-1]]
    gstart = np.flatnonzero(first)
    rank = np.arange(n) - np.repeat(gstart, np.diff(np.r_[gstart, n]))
    # order by (rank, position) stable -> rounds contiguous
    order2 = np.argsort(rank, kind="stable")
    rank_s = rank[order2]
    tok = order[order2]
    nr = np.bincount(rank_s)
    chunks_per_round = -(-nr // cap)
    starts = np.concatenate([[0], np.cumsum(chunks_per_round[:-1] * cap)])
    total_chunks = int(chunks_per_round.sum())
    pos = starts[rank_s] + (np.arange(n) - np.repeat(
        np.concatenate([[0], np.cumsum(nr[:-1])]), nr))
    cap_total = total_chunks * cap
    g = np.zeros(cap_total, np.int64)
    s = np.full(cap_total, PAD_ROW, np.int64)
    w = np.zeros(cap_total, np.float32)
    g[pos] = dsts[tok]
    s[pos] = srcs[tok]
    w[pos] = ws[tok]
    return g, s, w, total_chunks


def _prep_core(src, dst, w, s, ch):
    sel = (src >= NHALF) == bool(s)
    srcs = src[sel] - s * NHALF
    dsts = dst[sel]
    ws = w[sel]

    cap = ch * C
    g_all = np.zeros((2, cap), np.int64)
    s_all = np.full((2, cap), PAD_ROW, np.int64)
    w_all = np.zeros((2, cap), np.float32)
    for phase in range(2):
        pm = (dsts >= NHALF) == bool(phase)
        g, sarr, warr, nch = _round_pack(
            srcs[pm], dsts[pm] - phase * NHALF, ws[pm], C)
        assert nch <= ch, f"phase overflow: {nch} > {ch}"
        g_all[phase, :nch * C] = g
        s_all[phase, :nch * C] = sarr
        w_all[phase, :nch * C] = warr

    out = {}
    out["gidx"] = np.stack([_wrap16(g_all[p], ch, C) for p in range(2)])
    out["sidx"] = np.stack([_wrap16(s_all[p], ch, C) for p in range(2)])
    out["wl"] = np.ascontiguousarray(
        w_all.reshape(2, ch, C // 128, 128).transpose(0, 1, 3, 2)
    )
    return out


def _needed_chunks(src, dst, w):
    """Chunks per phase = sum over rounds r of ceil(#nodes-with-count>r / C)."""
    worst = 1
    for b in range(B):
        for s in range(2):
            sel = (src[b] >= NHALF) == bool(s)
            srcs = src[b][sel] - s * NHALF
            dsts = dst[b][sel]
            for phase in range(2):
                pm = (dsts >= NHALF) == bool(phase)
                ss = srcs[pm]
                cnts = np.bincount(ss, minlength=1)
                mx = int(cnts.max()) if cnts.size else 0
                rounds = np.array([(cnts > r).sum() for r in range(mx)])
                nch = int(np.sum(-(-rounds // C))) if mx else 1
                worst = max(worst, nch)
    return worst


def kernel(**inputs):
    H = np.ascontiguousarray(np.asarray(inputs["H"], np.float32))
    w = np.asarray(inputs["edge_w"], np.float32)
    src = np.asarray(inputs["edge_src"], np.int64)
    dst = np.asarray(inputs["edge_dst"], np.int64)

    ch = _needed_chunks(src, dst, w)
    nc = _get_compiled(ch)

    in_maps = []
    for core in range(8):
        b, s = core // 2, core % 2
        m = _prep_core(src[b], dst[b], w[b], s, ch)
        m["h"] = H[b]
        in_maps.append(m)

    trace = bool(int(os.environ.get("GNN_TRACE", "0")))
    res = run_bass_kernel_spmd(nc, in_maps, list(range(8)), trace=trace)
    LAST_RESULT["exec_time_ns"] = res.exec_time_ns
    LAST_RESULT["res"] = res

    out = np.empty((B, N, HS), np.float32)
    for core in range(8):
        b, s = core // 2, core % 2
        acc = res.results[core]["acc"].sum(axis=0)  # [2, 128, NGRP, HS]
        rows = acc.transpose(2, 0, 1, 3).reshape(-1, HS)[:NHALF]
        out[b, s * NHALF:(s + 1) * NHALF] = rows
    return out



# revision 5
# speedup vs baseline: 1.7392x; 1.7392x over previous
"""Neighbor aggregation (gnn message passing) Bass kernel for Trainium2.

out[b, i] = sum_{e: src[e]==i} w[e] * H[b, dst[e]]   (per batch b)

8 NeuronCores: core = 2*b + s handles batch b, src-half s (output rows
[s*25000, (s+1)*25000)).

Strategy ("bin-packed one-hot scatter"):
 - The only per-edge data-dependent hardware mechanism is the SWDGE
   dma_gather, whose Q7 descriptor generation costs ~7.8 ns/token and is the
   hard floor.  The previous kernel paid that floor TWICE (gather +
   dma_scatter_add).  This kernel pays it once: the scatter/segment-sum runs
   on the otherwise-idle Tensor engine as one-hot matmuls.
 - Host packs the 25000 output rows of each core into NBINS bins of <=128
   sources, balancing per-bin token counts for BOTH dst-half phases
   (<= TPB*128 tokens per bin per phase).  The resulting tile->bin map is a
   compile-time constant shared by all 8 SPMD cores; all per-core variation
   (gather indices, weights, slot-in-bin ids) is data.
 - Device: per 8192-token chunk: SWDGE dma_gather (HBM H rows -> SBUF
   token-major f32), DVE multiply by w (f32 -> bf16), then per 128-token
   tile: DVE is_equal against a constant iota row builds the one-hot
   [token, slot] matrix; TensorE matmul-accumulates one-hot.T @ msgs into a
   per-bin PSUM tile; after the bin's last tile, DVE adds PSUM into the SBUF
   accumulator [128 slots, NBINS, 64].  Host un-permutes (slot, bin) -> row.
 - Pad tokens gather row 0 with w=0 and slot=-1 (no one-hot match), so they
   are exact no-ops.  Both phases of a source share (slot, bin), so phase
   partials merge in the accumulator with no extra pass.
"""

import os
import sys

sys.path.insert(0, "/opt/trn_rl_repo")

import numpy as np
import ml_dtypes

import concourse.bacc as bacc
import concourse.mybir as mybir
import concourse.tile as tile
from concourse.bass_utils import run_bass_kernel_spmd

B, N, E, HS = 4, 50000, 800000, 64
NHALF = N // 2                  # 25000 output rows per core
CH = 8192                       # tokens per gather chunk
TPB = 7                         # tiles (of 128 tokens) per bin per phase

LAST_RESULT = {}


def build(nc, nbins, nch_per_phase):
    f32 = mybir.dt.float32
    bf16 = mybir.dt.bfloat16
    i16 = mybir.dt.int16
    nch = 2 * nch_per_phase
    tiles_per_phase = nch_per_phase * (CH // 128)
    real_tiles = nbins * TPB    # remaining tiles of the phase are dummies

    h_d = nc.dram_tensor("h", [N, HS], f32, kind="ExternalInput")
    gidx_d = nc.dram_tensor("gidx", [nch, 128, CH // 16], i16, kind="ExternalInput")
    wl_d = nc.dram_tensor("wl", [nch, 128, CH // 128], f32, kind="ExternalInput")
    scol_d = nc.dram_tensor("scol", [nch, 128, CH // 128], bf16, kind="ExternalInput")
    iotab_d = nc.dram_tensor("iotab", [128, 128], bf16, kind="ExternalInput")
    acc_d = nc.dram_tensor("acc", [128, nbins + 1, HS], f32, kind="ExternalOutput")

    with tile.TileContext(nc) as tc:
        with tc.tile_pool(name="res", bufs=1) as res, \
             tc.tile_pool(name="psum", bufs=6, space="PSUM") as pp, \
             tc.tile_pool(name="work", bufs=3) as wp, \
             tc.tile_pool(name="oh", bufs=4) as ohp:
            iotab = res.tile([128, 128], bf16, tag="iotab")
            nc.sync.dma_start(iotab[:], iotab_d[:])
            acc = res.tile([128, nbins + 1, HS], f32, tag="acc")
            nc.vector.memset(acc[:], 0.0)

            ps = None
            for c in range(nch):
                phase = c // nch_per_phase
                h_ap = h_d[:][phase * NHALF:(phase + 1) * NHALF, :]
                gi = wp.tile([128, CH // 16], i16, tag="gi")
                wl = wp.tile([128, CH // 128], f32, tag="wl")
                sc = wp.tile([128, CH // 128], bf16, tag="sc")
                nc.sync.dma_start(gi[:], gidx_d[c])
                nc.sync.dma_start(wl[:], wl_d[c])
                nc.sync.dma_start(sc[:], scol_d[c])

                msgs = wp.tile([128, CH // 128, HS], f32, tag="msgs")
                nc.gpsimd.dma_gather(
                    out_ap=msgs[:],
                    in_ap=h_ap,
                    idxs_ap=gi[:],
                    num_idxs=CH,
                    num_idxs_reg=CH,
                    elem_size=HS,
                    single_packet=False,
                )
                msgsb = wp.tile([128, CH // 128, HS], bf16, tag="msgsb")
                nc.vector.tensor_tensor(
                    out=msgsb[:],
                    in0=msgs[:],
                    in1=wl[:].unsqueeze(2).broadcast_to([128, CH // 128, HS]),
                    op=mybir.AluOpType.mult,
                )

                ntile = CH // 128
                for j0 in range(0, ntile, 4):
                    nb = min(4, ntile - j0)
                    oh = ohp.tile([128, 4, 128], bf16, tag="oh")
                    nc.vector.tensor_tensor(
                        out=oh[:, :nb],
                        in0=sc[:, j0:j0 + nb].unsqueeze(2).broadcast_to([128, nb, 128]),
                        in1=iotab[:].unsqueeze(1).broadcast_to([128, nb, 128]),
                        op=mybir.AluOpType.is_equal,
                    )
                    for j in range(j0, j0 + nb):
                        tau = (c % nch_per_phase) * ntile + j   # tile idx in phase
                        if tau < real_tiles:
                            bin_, pos = tau // TPB, tau % TPB
                            last = pos == TPB - 1
                        else:
                            bin_, pos = nbins, tau - real_tiles  # dummy bin
                            last = tau == tiles_per_phase - 1
                        if pos == 0:
                            ps = pp.tile([128, HS], f32, tag="ps")
                        nc.tensor.matmul(
                            ps[:], oh[:, j - j0], msgsb[:, j],
                            start=(pos == 0), stop=last,
                        )
                        if last:
                            nc.vector.tensor_tensor(
                                out=acc[:, bin_], in0=acc[:, bin_], in1=ps[:],
                                op=mybir.AluOpType.add,
                            )

            nc.sync.dma_start(acc_d[:], acc[:])
    return nc


_COMPILED = {}


def _get_compiled(nbins, nch_per_phase):
    key = (nbins, nch_per_phase)
    if key not in _COMPILED:
        nc = bacc.Bacc("TRN2", target_bir_lowering=False, debug=False)
        build(nc, nbins, nch_per_phase)
        nc.compile()
        _COMPILED[key] = nc
    return _COMPILED[key]


def _pack_bins(dA, dB, nbins, cap):
    """Assign each source to a bin s.t. per-bin source count <=128 and
    per-bin token sums <= cap in BOTH phases.  Returns (bin, slot) per
    source, or None if infeasible."""
    nsrc = dA.shape[0]
    order = np.argsort(-(dA + dB), kind="stable")
    loadA = np.zeros(nbins, np.int64)
    loadB = np.zeros(nbins, np.int64)
    cnt = np.zeros(nbins, np.int64)
    bin_of = np.empty(nsrc, np.int64)
    slot_of = np.empty(nsrc, np.int64)
    for s in order:
        headA = cap - loadA - dA[s]
        headB = cap - loadB - dB[s]
        score = np.minimum(headA, headB)
        score[cnt >= 128] = -1
        b = int(np.argmax(score))
        if score[b] < 0:
            return None
        bin_of[s] = b
        slot_of[s] = cnt[b]
        loadA[b] += dA[s]
        loadB[b] += dB[s]
        cnt[b] += 1
    return bin_of, slot_of


def _wrap16(idx, n):
    a = idx.reshape(n // 16, 16).T.astype(np.int16)   # [16, n//16]
    return np.ascontiguousarray(np.tile(a, (8, 1)))   # [128, n//16]


def _core_edges(src, dst, w, s):
    sel = (src >= NHALF) == bool(s)
    srcs = (src[sel] - s * NHALF).astype(np.int64)
    dsts = dst[sel].astype(np.int64)
    ws = w[sel].astype(np.float32)
    phase = (dsts >= NHALF).astype(np.int64)
    dloc = dsts - phase * NHALF
    return srcs, dloc, ws, phase


def _prep_core(srcs, dloc, ws, phase, bin_of, slot_of, nbins, nch_per_phase):
    """Build gidx/wl/scol chunk arrays for one core (batch half s)."""
    cap = TPB * 128
    ntok = nch_per_phase * CH
    g_all = np.zeros((2, ntok), np.int64)
    w_all = np.zeros((2, ntok), np.float32)
    s_all = np.full((2, ntok), -1.0, np.float32)

    for ph in range(2):
        m = phase == ph
        sp, dp, wp_ = srcs[m], dloc[m], ws[m]
        # order edges by bin: position = bin base + running offset within bin
        b = bin_of[sp]
        order = np.argsort(b, kind="stable")
        sp, dp, wp_, b = sp[order], dp[order], wp_[order], b[order]
        cnts = np.bincount(b, minlength=nbins)
        starts = np.concatenate([[0], np.cumsum(cnts[:-1])])
        offs = np.arange(sp.shape[0]) - np.repeat(starts, cnts)
        pos = b * cap + offs
        assert (offs < cap).all()
        g_all[ph, pos] = dp
        w_all[ph, pos] = wp_
        s_all[ph, pos] = slot_of[sp]

    gidx = np.stack([
        _wrap16(g_all[ph, c * CH:(c + 1) * CH], CH)
        for ph in range(2) for c in range(nch_per_phase)
    ])
    # token t of chunk -> [t % 128, t // 128]
    wl = np.ascontiguousarray(
        w_all.reshape(2 * nch_per_phase, CH // 128, 128).transpose(0, 2, 1))
    scol = np.ascontiguousarray(
        s_all.reshape(2 * nch_per_phase, CH // 128, 128).transpose(0, 2, 1)
    ).astype(ml_dtypes.bfloat16)
    return {"gidx": gidx, "wl": wl, "scol": scol}


def kernel(**inputs):
    H = np.ascontiguousarray(np.asarray(inputs["H"], np.float32))
    w = np.asarray(inputs["edge_w"], np.float32)
    src = np.asarray(inputs["edge_src"], np.int64)
    dst = np.asarray(inputs["edge_dst"], np.int64)

    cap = TPB * 128
    edges = []
    worst = 1
    for core in range(8):
        b, s = core // 2, core % 2
        srcs, dloc, ws, phase = _core_edges(src[b], dst[b], w[b], s)
        edges.append((srcs, dloc, ws, phase))
        worst = max(worst, int((phase == 0).sum()), int((phase == 1).sum()))

    # pack all cores; grow nbins until feasible everywhere
    nbins = max(-(-NHALF // 128), -(-int(worst * 1.01) // cap))
    nbins = -(-nbins // 4) * 4
    while True:
        metas = []
        for core in range(8):
            srcs, dloc, ws, phase = edges[core]
            dA = np.bincount(srcs[phase == 0], minlength=NHALF)
            dB = np.bincount(srcs[phase == 1], minlength=NHALF)
            res = _pack_bins(dA, dB, nbins, cap)
            if res is None:
                break
            metas.append(res)
        if len(metas) == 8:
            break
        nbins += 4
    nch_per_phase = -(-(nbins * cap) // CH)

    iotab = np.tile(np.arange(128), (128, 1)).astype(ml_dtypes.bfloat16)

    in_maps = []
    for core in range(8):
        b = core // 2
        srcs, dloc, ws, phase = edges[core]
        bin_of, slot_of = metas[core]
        m = _prep_core(srcs, dloc, ws, phase, bin_of, slot_of, nbins, nch_per_phase)
        m["h"] = H[b]
        m["iotab"] = iotab
        in_maps.append(m)

    nc = _get_compiled(nbins, nch_per_phase)
    trace = bool(int(os.environ.get("GNN_TRACE", "0")))
    res = run_bass_kernel_spmd(nc, in_maps, list(range(8)), trace=trace)
    LAST_RESULT["exec_time_ns"] = res.exec_time_ns
    LAST_RESULT["res"] = res

    out = np.empty((B, N, HS), np.float32)
    rows = np.arange(NHALF)
    for core in range(8):
        b, s = core // 2, core % 2
        bin_of, slot_of = metas[core]
        dump = res.results[core]["acc"]          # [128, nbins+1, 64]
        out[b, s * NHALF:(s + 1) * NHALF] = dump[slot_of[rows], bin_of[rows]]
    return out


# revision 7
# speedup vs baseline: 3.1043x; 1.7849x over previous
"""Neighbor aggregation (gnn message passing) Bass kernel for Trainium2.

out[b, i] = sum_{e: src[e]==i} w[e] * H[b, dst[e]]   (per batch b)

8 NeuronCores: core = 2*b + s handles batch b, src-half s (output rows
[s*25000, (s+1)*25000)).

Strategy ("bin-packed one-hot scatter"):
 - The only per-edge data-dependent hardware mechanism is the SWDGE
   dma_gather, whose Q7 descriptor generation costs ~7.8 ns/token and is the
   hard floor.  The previous kernel paid that floor TWICE (gather +
   dma_scatter_add).  This kernel pays it once: the scatter/segment-sum runs
   on the otherwise-idle Tensor engine as one-hot matmuls.
 - Host packs the 25000 output rows of each core into NBINS bins of <=128
   sources, balancing per-bin token counts for BOTH dst-half phases
   (<= TPB*128 tokens per bin per phase).  The resulting tile->bin map is a
   compile-time constant shared by all 8 SPMD cores; all per-core variation
   (gather indices, weights, slot-in-bin ids) is data.
 - Device: per 8192-token chunk: SWDGE dma_gather (HBM H rows -> SBUF
   token-major f32), DVE multiply by w (f32 -> bf16), then per 128-token
   tile: DVE is_equal against a constant iota row builds the one-hot
   [token, slot] matrix; TensorE matmul-accumulates one-hot.T @ msgs into a
   per-bin PSUM tile; after the bin's last tile, DVE adds PSUM into the SBUF
   accumulator [128 slots, NBINS, 64].  Host un-permutes (slot, bin) -> row.
 - Pad tokens gather row 0 with w=0 and slot=-1 (no one-hot match), so they
   are exact no-ops.  Both phases of a source share (slot, bin), so phase
   partials merge in the accumulator with no extra pass.
"""

import os
import sys

sys.path.insert(0, "/opt/trn_rl_repo")

import numpy as np
import ml_dtypes

import concourse.bacc as bacc
import concourse.mybir as mybir
import concourse.tile as tile
from concourse.bass_utils import run_bass_kernel_spmd

B, N, E, HS = 4, 50000, 800000, 64
NHALF = N // 2                  # 25000 output rows per core
CH = 8192                       # tokens per gather chunk
TPB = 7                         # tiles (of 128 tokens) per bin per phase

LAST_RESULT = {}


def build(nc, nbins, nch_per_phase):
    f32 = mybir.dt.float32
    bf16 = mybir.dt.bfloat16
    i16 = mybir.dt.int16
    nch = 2 * nch_per_phase
    tiles_per_phase = nch_per_phase * (CH // 128)
    real_tiles = nbins * TPB    # remaining tiles of the phase are dummies

    h_d = nc.dram_tensor("h", [N, HS], f32, kind="ExternalInput")
    gidx_d = nc.dram_tensor("gidx", [nch, 128, CH // 16], i16, kind="ExternalInput")
    wl_d = nc.dram_tensor("wl", [nch, 128, CH // 128], f32, kind="ExternalInput")
    scol_d = nc.dram_tensor("scol", [nch, 128, CH // 128], bf16, kind="ExternalInput")
    iotab_d = nc.dram_tensor("iotab", [128, 128], bf16, kind="ExternalInput")
    acc_d = nc.dram_tensor("acc", [128, nbins + 1, HS], f32, kind="ExternalOutput")

    with tile.TileContext(nc) as tc:
        with tc.tile_pool(name="res", bufs=1) as res, \
             tc.tile_pool(name="psum", bufs=6, space="PSUM") as pp, \
             tc.tile_pool(name="work", bufs=3) as wp, \
             tc.tile_pool(name="oh", bufs=4) as ohp:
            iotab = res.tile([128, 128], bf16, tag="iotab")
            nc.sync.dma_start(iotab[:], iotab_d[:])
            acc = res.tile([128, nbins + 1, HS], f32, tag="acc")
            nc.vector.memset(acc[:], 0.0)

            ps = None
            for c in range(nch):
                phase = c // nch_per_phase
                h_ap = h_d[:][phase * NHALF:(phase + 1) * NHALF, :]
                gi = wp.tile([128, CH // 16], i16, tag="gi")
                wl = wp.tile([128, CH // 128], f32, tag="wl")
                sc = wp.tile([128, CH // 128], bf16, tag="sc")
                nc.sync.dma_start(gi[:], gidx_d[c])
                nc.sync.dma_start(wl[:], wl_d[c])
                nc.sync.dma_start(sc[:], scol_d[c])

                msgs = wp.tile([128, CH // 128, HS], f32, tag="msgs")
                nc.gpsimd.dma_gather(
                    out_ap=msgs[:],
                    in_ap=h_ap,
                    idxs_ap=gi[:],
                    num_idxs=CH,
                    num_idxs_reg=CH,
                    elem_size=HS,
                    single_packet=False,
                    queue_num=c % 2,
                )
                msgsb = wp.tile([128, CH // 128, HS], bf16, tag="msgsb")
                nc.vector.tensor_tensor(
                    out=msgsb[:],
                    in0=msgs[:],
                    in1=wl[:].unsqueeze(2).broadcast_to([128, CH // 128, HS]),
                    op=mybir.AluOpType.mult,
                )

                ntile = CH // 128
                for j0 in range(0, ntile, 4):
                    nb = min(4, ntile - j0)
                    oh = ohp.tile([128, 4, 128], bf16, tag="oh")
                    nc.vector.tensor_tensor(
                        out=oh[:, :nb],
                        in0=sc[:, j0:j0 + nb].unsqueeze(2).broadcast_to([128, nb, 128]),
                        in1=iotab[:].unsqueeze(1).broadcast_to([128, nb, 128]),
                        op=mybir.AluOpType.is_equal,
                    )
                    for j in range(j0, j0 + nb):
                        tau = (c % nch_per_phase) * ntile + j   # tile idx in phase
                        if tau < real_tiles:
                            bin_, pos = tau // TPB, tau % TPB
                            last = pos == TPB - 1
                        else:
                            bin_, pos = nbins, tau - real_tiles  # dummy bin
                            last = tau == tiles_per_phase - 1
                        if pos == 0:
                            ps = pp.tile([128, HS], f32, tag="ps")
                        nc.tensor.matmul(
                            ps[:], oh[:, j - j0], msgsb[:, j],
                            start=(pos == 0), stop=last,
                        )
                        if last:
                            nc.vector.tensor_tensor(
                                out=acc[:, bin_], in0=acc[:, bin_], in1=ps[:],
                                op=mybir.AluOpType.add,
                            )

            nc.sync.dma_start(acc_d[:], acc[:])
    return nc


_COMPILED = {}


def _get_compiled(nbins, nch_per_phase):
    key = (nbins, nch_per_phase)
    if key not in _COMPILED:
        nc = bacc.Bacc(
            "TRN2", target_bir_lowering=False, debug=False, num_swdge_queues=2
        )
        build(nc, nbins, nch_per_phase)
        nc.compile()
        _COMPILED[key] = nc
    return _COMPILED[key]


def _pack_bins(dA, dB, nbins, cap):
    """Assign each source to a bin s.t. per-bin source count <=128 and
    per-bin token sums <= cap in BOTH phases.  Returns (bin, slot) per
    source, or None if infeasible."""
    nsrc = dA.shape[0]
    order = np.argsort(-(dA + dB), kind="stable")
    loadA = np.zeros(nbins, np.int64)
    loadB = np.zeros(nbins, np.int64)
    cnt = np.zeros(nbins, np.int64)
    bin_of = np.empty(nsrc, np.int64)
    slot_of = np.empty(nsrc, np.int64)
    for s in order:
        headA = cap - loadA - dA[s]
        headB = cap - loadB - dB[s]
        score = np.minimum(headA, headB)
        score[cnt >= 128] = -1
        b = int(np.argmax(score))
        if score[b] < 0:
            return None
        bin_of[s] = b
        slot_of[s] = cnt[b]
        loadA[b] += dA[s]
        loadB[b] += dB[s]
        cnt[b] += 1
    return bin_of, slot_of


def _wrap16(idx, n):
    a = idx.reshape(n // 16, 16).T.astype(np.int16)   # [16, n//16]
    return np.ascontiguousarray(np.tile(a, (8, 1)))   # [128, n//16]


def _core_edges(src, dst, w, s):
    sel = (src >= NHALF) == bool(s)
    srcs = (src[sel] - s * NHALF).astype(np.int64)
    dsts = dst[sel].astype(np.int64)
    ws = w[sel].astype(np.float32)
    phase = (dsts >= NHALF).astype(np.int64)
    dloc = dsts - phase * NHALF
    return srcs, dloc, ws, phase


def _prep_core(srcs, dloc, ws, phase, bin_of, slot_of, nbins, nch_per_phase):
    """Build gidx/wl/scol chunk arrays for one core (batch half s)."""
    cap = TPB * 128
    ntok = nch_per_phase * CH
    g_all = np.zeros((2, ntok), np.int64)
    w_all = np.zeros((2, ntok), np.float32)
    s_all = np.full((2, ntok), -1.0, np.float32)

    for ph in range(2):
        m = phase == ph
        sp, dp, wp_ = srcs[m], dloc[m], ws[m]
        # order edges by bin: position = bin base + running offset within bin
        b = bin_of[sp]
        order = np.argsort(b, kind="stable")
        sp, dp, wp_, b = sp[order], dp[order], wp_[order], b[order]
        cnts = np.bincount(b, minlength=nbins)
        starts = np.concatenate([[0], np.cumsum(cnts[:-1])])
        offs = np.arange(sp.shape[0]) - np.repeat(starts, cnts)
        pos = b * cap + offs
        assert (offs < cap).all()
        g_all[ph, pos] = dp
        w_all[ph, pos] = wp_
        s_all[ph, pos] = slot_of[sp]

    gidx = np.stack([
        _wrap16(g_all[ph, c * CH:(c + 1) * CH], CH)
        for ph in range(2) for c in range(nch_per_phase)
    ])
    # token t of chunk -> [t % 128, t // 128]
    wl = np.ascontiguousarray(
        w_all.reshape(2 * nch_per_phase, CH // 128, 128).transpose(0, 2, 1))
    scol = np.ascontiguousarray(
        s_all.reshape(2 * nch_per_phase, CH // 128, 128).transpose(0, 2, 1)
    ).astype(ml_dtypes.bfloat16)
    return {"gidx": gidx, "wl": wl, "scol": scol}


def kernel(**inputs):
    H = np.ascontiguousarray(np.asarray(inputs["H"], np.float32))
    w = np.asarray(inputs["edge_w"], np.float32)
    src = np.asarray(inputs["edge_src"], np.int64)
    dst = np.asarray(inputs["edge_dst"], np.int64)

    cap = TPB * 128
    edges = []
    worst = 1
    for core in range(8):
        b, s = core // 2, core % 2
        srcs, dloc, ws, phase = _core_edges(src[b], dst[b], w[b], s)
        edges.append((srcs, dloc, ws, phase))
        worst = max(worst, int((phase == 0).sum()), int((phase == 1).sum()))

    # pack all cores; grow nbins until feasible everywhere
    nbins = max(-(-NHALF // 128), -(-int(worst * 1.01) // cap))
    nbins = -(-nbins // 4) * 4
    while True:
        metas = []
        for core in range(8):
            srcs, dloc, ws, phase = edges[core]
            dA = np.bincount(srcs[phase == 0], minlength=NHALF)
            dB = np.bincount(srcs[phase == 1], minlength=NHALF)
            res = _pack_bins(dA, dB, nbins, cap)
            if res is None:
                break
            metas.append(res)
        if len(metas) == 8:
            break
        nbins += 4
    nch_per_phase = -(-(nbins * cap) // CH)

    iotab = np.tile(np.arange(128), (128, 1)).astype(ml_dtypes.bfloat16)

    in_maps = []
    for core in range(8):
        b = core // 2
        srcs, dloc, ws, phase = edges[core]
        bin_of, slot_of = metas[core]
        m = _prep_core(srcs, dloc, ws, phase, bin_of, slot_of, nbins, nch_per_phase)
        m["h"] = H[b]
        m["iotab"] = iotab
        in_maps.append(m)

    nc = _get_compiled(nbins, nch_per_phase)
    trace = bool(int(os.environ.get("GNN_TRACE", "0")))
    res = run_bass_kernel_spmd(nc, in_maps, list(range(8)), trace=trace)
    LAST_RESULT["exec_time_ns"] = res.exec_time_ns
    LAST_RESULT["res"] = res

    out = np.empty((B, N, HS), np.float32)
    rows = np.arange(NHALF)
    for core in range(8):
        b, s = core // 2, core % 2
        bin_of, slot_of = metas[core]
        dump = res.results[core]["acc"]          # [128, nbins+1, 64]
        out[b, s * NHALF:(s + 1) * NHALF] = dump[slot_of[rows], bin_of[rows]]
    return out


# revision 8
# speedup vs baseline: 4.8011x; 1.5466x over previous
"""Neighbor aggregation (gnn message passing) Bass kernel for Trainium2.

out[b, i] = sum_{e: src[e]==i} w[e] * H[b, dst[e]]   (per batch b)

8 NeuronCores: core = 2*b + s handles batch b, src-half s (output rows
[s*25000, (s+1)*25000)).

Strategy ("bin-packed one-hot scatter"):
 - The only per-edge data-dependent hardware mechanism is the SWDGE
   dma_gather, whose Q7 descriptor generation costs ~7.8 ns/token and is the
   hard floor.  The previous kernel paid that floor TWICE (gather +
   dma_scatter_add).  This kernel pays it once: the scatter/segment-sum runs
   on the otherwise-idle Tensor engine as one-hot matmuls.
 - Host packs the 25000 output rows of each core into NBINS bins of <=128
   sources, balancing per-bin token counts for BOTH dst-half phases
   (<= TPB*128 tokens per bin per phase).  The resulting tile->bin map is a
   compile-time constant shared by all 8 SPMD cores; all per-core variation
   (gather indices, weights, slot-in-bin ids) is data.
 - Device: per 8192-token chunk: SWDGE dma_gather (HBM H rows -> SBUF
   token-major f32), DVE multiply by w (f32 -> bf16), then per 128-token
   tile: DVE is_equal against a constant iota row builds the one-hot
   [token, slot] matrix; TensorE matmul-accumulates one-hot.T @ msgs into a
   per-bin PSUM tile; after the bin's last tile, DVE adds PSUM into the SBUF
   accumulator [128 slots, NBINS, 64].  Host un-permutes (slot, bin) -> row.
 - Pad tokens gather row 0 with w=0 and slot=-1 (no one-hot match), so they
   are exact no-ops.  Both phases of a source share (slot, bin), so phase
   partials merge in the accumulator with no extra pass.
"""

import os
import sys

sys.path.insert(0, "/opt/trn_rl_repo")

import numpy as np
import ml_dtypes

import concourse.bacc as bacc
import concourse.mybir as mybir
import concourse.tile as tile
from concourse.bass_utils import run_bass_kernel_spmd

B, N, E, HS = 4, 50000, 800000, 64
NHALF = N // 2                  # 25000 output rows per core
CH = 8192                       # tokens per gather chunk
TPB = 7                         # tiles (of 128 tokens) per bin per phase

LAST_RESULT = {}


def build(nc, nbins, nch_per_phase):
    f32 = mybir.dt.float32
    bf16 = mybir.dt.bfloat16
    i16 = mybir.dt.int16
    nch = 2 * nch_per_phase
    tiles_per_phase = nch_per_phase * (CH // 128)
    real_tiles = nbins * TPB    # remaining tiles of the phase are dummies

    h_d = nc.dram_tensor("h", [N, HS], f32, kind="ExternalInput")
    gidx_d = nc.dram_tensor("gidx", [nch, 128, CH // 16], i16, kind="ExternalInput")
    wl_d = nc.dram_tensor("wl", [nch, 128, CH // 128], f32, kind="ExternalInput")
    scol_d = nc.dram_tensor("scol", [nch, 128, CH // 128], bf16, kind="ExternalInput")
    iotab_d = nc.dram_tensor("iotab", [128, 128], bf16, kind="ExternalInput")
    acc_d = nc.dram_tensor("acc", [128, nbins + 1, HS], f32, kind="ExternalOutput")

    with tile.TileContext(nc) as tc:
        with tc.tile_pool(name="res", bufs=1) as res, \
             tc.tile_pool(name="psum", bufs=6, space="PSUM") as pp, \
             tc.tile_pool(name="work", bufs=3) as wp, \
             tc.tile_pool(name="oh", bufs=4) as ohp:
            iotab = res.tile([128, 128], bf16, tag="iotab")
            nc.sync.dma_start(iotab[:], iotab_d[:])
            acc = res.tile([128, nbins + 1, HS], f32, tag="acc")
            nc.vector.memset(acc[:], 0.0)

            ps = None
            for c in range(nch):
                phase = c // nch_per_phase
                h_ap = h_d[:][phase * NHALF:(phase + 1) * NHALF, :]
                gi = wp.tile([128, CH // 16], i16, tag="gi")
                wl = wp.tile([128, CH // 128], f32, tag="wl")
                sc = wp.tile([128, CH // 128], bf16, tag="sc")
                nc.sync.dma_start(gi[:], gidx_d[c])
                nc.sync.dma_start(wl[:], wl_d[c])
                nc.sync.dma_start(sc[:], scol_d[c])

                msgs = wp.tile([128, CH // 128, HS], f32, tag="msgs")
                nc.gpsimd.dma_gather(
                    out_ap=msgs[:],
                    in_ap=h_ap,
                    idxs_ap=gi[:],
                    num_idxs=CH,
                    num_idxs_reg=CH,
                    elem_size=HS,
                    single_packet=False,
                    queue_num=c % 4,
                )
                msgsb = wp.tile([128, CH // 128, HS], bf16, tag="msgsb")
                nc.vector.tensor_tensor(
                    out=msgsb[:],
                    in0=msgs[:],
                    in1=wl[:].unsqueeze(2).broadcast_to([128, CH // 128, HS]),
                    op=mybir.AluOpType.mult,
                )

                ntile = CH // 128
                for j0 in range(0, ntile, 4):
                    nb = min(4, ntile - j0)
                    oh = ohp.tile([128, 4, 128], bf16, tag="oh")
                    nc.vector.tensor_tensor(
                        out=oh[:, :nb],
                        in0=sc[:, j0:j0 + nb].unsqueeze(2).broadcast_to([128, nb, 128]),
                        in1=iotab[:].unsqueeze(1).broadcast_to([128, nb, 128]),
                        op=mybir.AluOpType.is_equal,
                    )
                    for j in range(j0, j0 + nb):
                        tau = (c % nch_per_phase) * ntile + j   # tile idx in phase
                        if tau < real_tiles:
                            bin_, pos = tau // TPB, tau % TPB
                            last = pos == TPB - 1
                        else:
                            bin_, pos = nbins, tau - real_tiles  # dummy bin
                            last = tau == tiles_per_phase - 1
                        if pos == 0:
                            ps = pp.tile([128, HS], f32, tag="ps")
                        nc.tensor.matmul(
                            ps[:], oh[:, j - j0], msgsb[:, j],
                            start=(pos == 0), stop=last,
                        )
                        if last:
                            nc.vector.tensor_tensor(
                                out=acc[:, bin_], in0=acc[:, bin_], in1=ps[:],
                                op=mybir.AluOpType.add,
                            )

            nc.sync.dma_start(acc_d[:], acc[:])
    return nc


_COMPILED = {}


def _get_compiled(nbins, nch_per_phase):
    key = (nbins, nch_per_phase)
    if key not in _COMPILED:
        nc = bacc.Bacc(
            "TRN2", target_bir_lowering=False, debug=False, num_swdge_queues=4
        )
        build(nc, nbins, nch_per_phase)
        nc.compile()
        _COMPILED[key] = nc
    return _COMPILED[key]


def _pack_bins(dA, dB, nbins, cap):
    """Assign each source to a bin s.t. per-bin source count <=128 and
    per-bin token sums <= cap in BOTH phases.  Returns (bin, slot) per
    source, or None if infeasible."""
    nsrc = dA.shape[0]
    order = np.argsort(-(dA + dB), kind="stable")
    loadA = np.zeros(nbins, np.int64)
    loadB = np.zeros(nbins, np.int64)
    cnt = np.zeros(nbins, np.int64)
    bin_of = np.empty(nsrc, np.int64)
    slot_of = np.empty(nsrc, np.int64)
    for s in order:
        headA = cap - loadA - dA[s]
        headB = cap - loadB - dB[s]
        score = np.minimum(headA, headB)
        score[cnt >= 128] = -1
        b = int(np.argmax(score))
        if score[b] < 0:
            return None
        bin_of[s] = b
        slot_of[s] = cnt[b]
        loadA[b] += dA[s]
        loadB[b] += dB[s]
        cnt[b] += 1
    return bin_of, slot_of


def _wrap16(idx, n):
    a = idx.reshape(n // 16, 16).T.astype(np.int16)   # [16, n//16]
    return np.ascontiguousarray(np.tile(a, (8, 1)))   # [128, n//16]


def _core_edges(src, dst, w, s):
    sel = (src >= NHALF) == bool(s)
    srcs = (src[sel] - s * NHALF).astype(np.int64)
    dsts = dst[sel].astype(np.int64)
    ws = w[sel].astype(np.float32)
    phase = (dsts >= NHALF).astype(np.int64)
    dloc = dsts - phase * NHALF
    return srcs, dloc, ws, phase


def _prep_core(srcs, dloc, ws, phase, bin_of, slot_of, nbins, nch_per_phase):
    """Build gidx/wl/scol chunk arrays for one core (batch half s)."""
    cap = TPB * 128
    ntok = nch_per_phase * CH
    g_all = np.zeros((2, ntok), np.int64)
    w_all = np.zeros((2, ntok), np.float32)
    s_all = np.full((2, ntok), -1.0, np.float32)

    for ph in range(2):
        m = phase == ph
        sp, dp, wp_ = srcs[m], dloc[m], ws[m]
        # order edges by bin: position = bin base + running offset within bin
        b = bin_of[sp]
        order = np.argsort(b, kind="stable")
        sp, dp, wp_, b = sp[order], dp[order], wp_[order], b[order]
        cnts = np.bincount(b, minlength=nbins)
        starts = np.concatenate([[0], np.cumsum(cnts[:-1])])
        offs = np.arange(sp.shape[0]) - np.repeat(starts, cnts)
        pos = b * cap + offs
        assert (offs < cap).all()
        g_all[ph, pos] = dp
        w_all[ph, pos] = wp_
        s_all[ph, pos] = slot_of[sp]

    gidx = np.stack([
        _wrap16(g_all[ph, c * CH:(c + 1) * CH], CH)
        for ph in range(2) for c in range(nch_per_phase)
    ])
    # token t of chunk -> [t % 128, t // 128]
    wl = np.ascontiguousarray(
        w_all.reshape(2 * nch_per_phase, CH // 128, 128).transpose(0, 2, 1))
    scol = np.ascontiguousarray(
        s_all.reshape(2 * nch_per_phase, CH // 128, 128).transpose(0, 2, 1)
    ).astype(ml_dtypes.bfloat16)
    return {"gidx": gidx, "wl": wl, "scol": scol}


def kernel(**inputs):
    H = np.ascontiguousarray(np.asarray(inputs["H"], np.float32))
    w = np.asarray(inputs["edge_w"], np.float32)
    src = np.asarray(inputs["edge_src"], np.int64)
    dst = np.asarray(inputs["edge_dst"], np.int64)

    cap = TPB * 128
    edges = []
    worst = 1
    for core in range(8):
        b, s = core // 2, core % 2
        srcs, dloc, ws, phase = _core_edges(src[b], dst[b], w[b], s)
        edges.append((srcs, dloc, ws, phase))
        worst = max(worst, int((phase == 0).sum()), int((phase == 1).sum()))

    # pack all cores; grow nbins until feasible everywhere
    nbins = max(-(-NHALF // 128), -(-int(worst * 1.01) // cap))
    nbins = -(-nbins // 4) * 4
    while True:
        metas = []
        for core in range(8):
            srcs, dloc, ws, phase = edges[core]
            dA = np.bincount(srcs[phase == 0], minlength=NHALF)
            dB = np.bincount(srcs[phase == 1], minlength=NHALF)
            res = _pack_bins(dA, dB, nbins, cap)
            if res is None:
                break
            metas.append(res)
        if len(metas) == 8:
            break
        nbins += 4
    nch_per_phase = -(-(nbins * cap) // CH)

    iotab = np.tile(np.arange(128), (128, 1)).astype(ml_dtypes.bfloat16)

    in_maps = []
    for core in range(8):
        b = core // 2
        srcs, dloc, ws, phase = edges[core]
        bin_of, slot_of = metas[core]
        m = _prep_core(srcs, dloc, ws, phase, bin_of, slot_of, nbins, nch_per_phase)
        m["h"] = H[b]
        m["iotab"] = iotab
        in_maps.append(m)

    nc = _get_compiled(nbins, nch_per_phase)
    trace = bool(int(os.environ.get("GNN_TRACE", "0")))
    res = run_bass_kernel_spmd(nc, in_maps, list(range(8)), trace=trace)
    LAST_RESULT["exec_time_ns"] = res.exec_time_ns
    LAST_RESULT["res"] = res

    out = np.empty((B, N, HS), np.float32)
    rows = np.arange(NHALF)
    for core in range(8):
        b, s = core // 2, core % 2
        bin_of, slot_of = metas[core]
        dump = res.results[core]["acc"]          # [128, nbins+1, 64]
        out[b, s * NHALF:(s + 1) * NHALF] = dump[slot_of[rows], bin_of[rows]]
    return out
